# revision 15
# baseline (speedup 1.0000x reference)
"""Canny edge detector (32,1,1024,1024) on 8 Trainium2 NeuronCores.

v2 strategy (per core: 4 images, data-parallel over batch):
  - Row-tiles of 128 partitions (120-row output tiles + 4-row halo wrap).
  - Blur: 5 dx-shifted band matmuls x {fp16 hi + bf16 lo} weight pair
    (bf16 lo needs no 2^12 rescale: bf16 exponent range covers ~1e-5
    residuals), rhs = fp16 xq for both. fp32 PSUM.
  - bh = fp16(blur) evac on ACT; bl computed on the PE (psum -= I@bh)
    and evac'd on ACT -> exact fp16 hi/lo pair without a DVE subtract.
  - Sobel: integer fp16 bands on {bh, bl}, PSUM fp32.
  - NMS: mag/direction-mask fused DVE customs reading gx(SBUF)/gy(PSUM);
    row-shifted U/D via fp32 identity-band matmuls into dedicated 1-bank
    PSUM tiles (pu/pd) so the blur/sobel PSUM slots free early and
    adjacent tiles overlap; pair maxes + 3 predicated overwrites +
    fused keep threshold on DVE.
  - Engine placement tuned to measured costs: ACT takes all evacuations
    and pad copies, DVE keeps only tensor-tensor/custom/predicated work.
"""

import os
import numpy as np

import concourse.bacc as bacc
import concourse.tile as tile
import concourse.mybir as mybir
from concourse import bass_utils
from concourse import dve_ops as _DO
from concourse.dve_spec import (
    Spec, Src0, Src1, C0, C1, C2, Zero, One, maxx, select, Bin, AluOp,
    lower as _dve_lower,
)
from concourse.dve_uop import DveOpSpec as _DveOpSpec


def _register_custom_op(name, body, ref):
    """Runtime-register a fused DVE op (sha self-computed, v3/TRN2)."""
    if name in _DO._SUB_OPCODE_FOR_NAME:
        return next(op for op in _DO.OPS if op.name == name)
    op = _DO.DveOp(name, Spec(body=body, reference=ref), subdim=False, uops_sha={})
    _DO.OPS.append(op)
    _DO.CUSTOM_DVE_SPECS[name] = op.spec
    _DO._SUB_OPCODE_FOR_NAME[name] = _DO._CUSTOM_DVE_ROW_BASE + len(_DO.OPS) - 1
    for ver in ("v3",):
        compiled = _DveOpSpec(
            name=name,
            opcode=_DO.get_dve_sub_opcode(name),
            uops=_dve_lower(op.spec, ver=ver),
            rd1_en=True,
        )
        op.uops_sha[ver] = compiled.sha(ver)
    return op


_ABS0 = maxx(Src0, Zero - Src0)
_ABS1 = maxx(Src1, Zero - Src1)
OP_MAG = _register_custom_op(
    "CANNY_MAG", _ABS0 + _ABS1,
    lambda in0, in1, s0, s1, imm2: np.abs(in0) + np.abs(in1))
# bin code: 0 = E/W (|gy|*T2 < |gx|), 2 = N/S (|gx|*T2 <= |gy|),
# 1 = NE/SW diag (gx*gy > 0), 3 = NW/SE diag. in0 = gx, in1 = gy.
OP_POS = _register_custom_op(
    "CANNY_POS", (Src0 * Src1) > Zero,
    lambda in0, in1, s0, s1, imm2: (in0 * in1 > 0).astype(np.float32))
OP_C0 = _register_custom_op(
    "CANNY_C0", (_ABS0 * C0) < _ABS1,
    lambda in0, in1, s0, s1, imm2: (np.abs(in0) * s0 < np.abs(in1)).astype(np.float32))
OP_C2 = _register_custom_op(
    "CANNY_C2", (_ABS1 * C0) <= _ABS0,
    lambda in0, in1, s0, s1, imm2: (np.abs(in1) * s0 <= np.abs(in0)).astype(np.float32))
OP_KEEP = _register_custom_op(
    "CANNY_KEEP", (Src0 >= Src1) & (Src0 > C0),
    lambda in0, in1, s0, s1, imm2: ((in0 >= in1) & (in0 > s0)).astype(np.float32))

H = W = 1024
NCORES = 8
IMGS_PER_CORE = 4
TILE_STARTS = [0, 120, 240, 360, 480, 600, 720, 840, 904]
NKEEP = 120
T2 = float(np.float32(1.0 + np.sqrt(2.0)))  # tan(67.5 deg)
THR = 20.0

# ----------------------------------------------------------------------------
# band construction (host, float64 -> float32 taps identical to the reference)
# ----------------------------------------------------------------------------

def _gauss5_f64():
    x = np.arange(5.0) - 2.0
    k = np.exp(-(x ** 2) / (2.0 * 9.0))
    return k / k.sum()

G64 = _gauss5_f64()
COL_SMOOTH = np.array([1.0, 2.0, 1.0])   # sobel smoothing (column filter of Sx)
COL_DIFF = np.array([-1.0, 0.0, 1.0])    # sobel derivative (column filter of Sy)


def _bf16(a):
    a32 = np.asarray(a, np.float32)
    u = a32.view(np.uint32)
    # round-to-nearest-even on the upper 16 bits
    u = (u + 0x7FFF + ((u >> 16) & 1)).astype(np.uint32) & np.uint32(0xFFFF0000)
    return u.view(np.float32)


def _row_of(p, r0):
    return r0 + (p if p < 124 else p - 128)


def _part_of(v, r0):
    d = v - r0
    assert -4 <= d < 124, (v, r0)
    return d if d >= 0 else d + 128


def _reflect(v):
    if v < 0:
        return -v
    if v > H - 1:
        return 2 * (H - 1) - v
    return v


def _blur_bands(r0):
    """(5 fp16 hi, 5 bf16-lo-as-f32) [128,128] matrices for dx=-2..2."""
    his, los = [], []
    for dx in range(-2, 3):
        B = np.zeros((128, 128), np.float64)
        for j in range(128):
            d = j if j < 124 else j - 128
            if not (-2 <= d <= 121):
                continue
            v = _row_of(j, r0)
            if not (0 <= v <= H - 1):
                continue
            for dz in range(-2, 3):
                u = v + dz  # virtual x row; tile holds r0-4..r0+123 virtually
                B[_part_of(u, r0), j] += G64[dz + 2] * G64[dx + 2]
        B32 = B.astype(np.float32)
        BH = B32.astype(np.float16)
        BL = _bf16(B32.astype(np.float64) - BH.astype(np.float64))
        his.append(BH)
        los.append(BL)
    return his, los


def _sobel_bands(r0):
    """5 fp16 [128,128] integer matrices: SGX(dx=-1), SGX(+1), SGY(-1), SGY(0), SGY(+1)."""
    mats = []
    for colfilt, rowtaps in ((COL_SMOOTH, [-1.0, 1.0]), (COL_DIFF, [1.0, 2.0, 1.0])):
        if colfilt is COL_SMOOTH:
            dxs = [-1, 1]
        else:
            dxs = [-1, 0, 1]
        for idx, dx in enumerate(dxs):
            rt = rowtaps[idx] if colfilt is COL_SMOOTH else rowtaps[dx + 1]
            B = np.zeros((128, 128), np.float64)
            for j in range(128):
                d = j if j < 124 else j - 128
                if not (-1 <= d <= 120):
                    continue
                v = _row_of(j, r0)
                if not (0 <= v <= H - 1):
                    continue
                for dz in (-1, 0, 1):
                    w = colfilt[dz + 1]
                    if w == 0.0:
                        continue
                    u = _reflect(v + dz)  # reflect-101 on BLUR rows
                    B[_part_of(u, r0), j] += rt * w
            mats.append(B.astype(np.float16))
    return mats


def _shift_mats():
    SUP = np.zeros((128, 128), np.float16)  # U[j] = mag[j-1 (mod 128)]
    SDN = np.zeros((128, 128), np.float16)  # D[j] = mag[j+1]
    NEGI = np.zeros((128, 128), np.float16)
    for j in range(128):
        SUP[(j - 1) % 128, j] = 1.0
        NEGI[j, j] = -1.0
    for j in range(127):
        SDN[j + 1, j] = 1.0
    return SUP, SDN, NEGI


def _pack_weights():
    """wt16 [128, 33*128] f16 (per class: 5 blur-hi + 5 sobel; then SUP, SDN, NEGI),
    wlo [128, 15*128] f32-encoded-bf16 column slices."""
    mats16 = []
    matslo = []
    for r0 in (TILE_STARTS[0], TILE_STARTS[1], TILE_STARTS[-1]):
        his, los = _blur_bands(r0)
        mats16.extend(his)
        mats16.extend(_sobel_bands(r0))
        matslo.extend(los)
    SUP, SDN, NEGI = _shift_mats()
    mats16.extend([SUP, SDN, NEGI])
    wt16 = np.stack([m.astype(np.float16) for m in mats16], 0)      # [33,128,128]
    wt16 = np.transpose(wt16, (1, 0, 2)).reshape(128, -1).copy()
    wlo32 = np.stack(matslo, 0)                                     # [15,128,128] f32
    wlo32 = np.transpose(wlo32, (1, 0, 2)).reshape(128, -1).copy()
    wsh = np.stack([SUP.astype(np.float32), SDN.astype(np.float32)], 0)
    wsh = np.transpose(wsh, (1, 0, 2)).reshape(128, -1).copy()
    import ml_dtypes
    return (wt16.astype(np.float16), wlo32.astype(ml_dtypes.bfloat16),
            wsh.astype(np.float32))


def _tile_class(ti):
    if ti == 0:
        return 0
    if ti == len(TILE_STARTS) - 1:
        return 2
    return 1


# ----------------------------------------------------------------------------
# kernel builder
# ----------------------------------------------------------------------------

def build_kernel(n_img=IMGS_PER_CORE, tiles=None, dump=False, repeat=1, ablate=()):
    if tiles is None:
        tiles = list(range(len(TILE_STARTS)))
    AL = mybir.AluOpType
    f32, f16, bf16, i32 = (mybir.dt.float32, mybir.dt.float16,
                           mybir.dt.bfloat16, mybir.dt.int32)

    nc = bacc.Bacc("TRN2", target_bir_lowering=False, debug=False)
    img_d = nc.dram_tensor("image", [n_img, H, W], f32, kind="ExternalInput").ap()
    wt16_d = nc.dram_tensor("wt16", [128, 33 * 128], f16, kind="ExternalInput").ap()
    wlo_d = nc.dram_tensor("wlo", [128, 15 * 128], bf16, kind="ExternalInput").ap()
    wsh_d = nc.dram_tensor("wsh", [128, 2 * 128], f32, kind="ExternalInput").ap()
    out_d = nc.dram_tensor("out", [n_img, H, W], f32, kind="ExternalOutput").ap()
    if dump:
        dmp = {k: nc.dram_tensor("dbg_" + k, [128, W + 4], f32, kind="ExternalOutput").ap()
               for k in ["xq", "bh", "bl", "gxs", "mag", "usb", "dsb", "M", "bin"]}

    with tile.TileContext(nc) as tc:
        with (
            tc.tile_pool(name="wts", bufs=1) as wp,
            tc.tile_pool(name="io", bufs=4) as iop,
            tc.tile_pool(name="mid", bufs=3) as mp,
            tc.tile_pool(name="nms", bufs=2) as np_,
            tc.tile_pool(name="ps", bufs=1, space="PSUM") as pp,
        ):
            wt16 = wp.tile([128, 33 * 128], f16)
            wlo = wp.tile([128, 15 * 128], bf16)
            wsh = wp.tile([128, 2 * 128], f32)
            nc.sync.dma_start(out=wt16[:, :], in_=wt16_d[:, :])
            nc.sync.dma_start(out=wlo[:, :], in_=wlo_d[:, :])
            nc.sync.dma_start(out=wsh[:, :], in_=wsh_d[:, :])

            def m16(c, k):  # fp16 matrix k (0..9) of tile-class c
                s = (c * 10 + k) * 128
                return wt16[:, s:s + 128]

            def msh(k):  # 0 = SUP, 1 = SDN, 2 = NEGI (fp16)
                s = (30 + k) * 128
                return wt16[:, s:s + 128]

            def msh32(k):  # 0 = SUP, 1 = SDN (fp32)
                return wsh[:, k * 128:(k + 1) * 128]

            def mlo(c, k):  # bf16 blur-lo matrix k (0..4) of tile-class c
                s = (c * 5 + k) * 128
                return wlo[:, s:s + 128]

            for _rep in range(repeat):
              for i in range(n_img):
                for ti in tiles:
                    r0 = TILE_STARTS[ti]
                    cls = _tile_class(ti)

                    # ---- load image tile (fp32), incl. reflected pad cols ------
                    img_t = iop.tile([128, W + 4], f32, tag="img")
                    if ti == len(TILE_STARTS) - 1:
                        nc.sync.dma_start(out=img_t[0:120, 2:W + 2],
                                          in_=img_d[i, r0:r0 + 120, :])
                        for k in range(4):  # virtual rows 1024..1027 = 1022..1019
                            nc.sync.dma_start(out=img_t[120 + k:121 + k, 2:W + 2],
                                              in_=img_d[i, 1022 - k:1023 - k, :])
                    else:
                        nc.sync.dma_start(out=img_t[0:124, 2:W + 2],
                                          in_=img_d[i, r0:r0 + 124, :])
                    if ti == 0:
                        for k in range(4):  # virtual rows -4..-1 = rows 4,3,2,1
                            nc.sync.dma_start(out=img_t[124 + k:125 + k, 2:W + 2],
                                              in_=img_d[i, 4 - k:5 - k, :])
                    else:
                        nc.sync.dma_start(out=img_t[124:128, 2:W + 2],
                                          in_=img_d[i, r0 - 4:r0, :])
                    # reflected pad columns (ACT: full 128-partition col copies)
                    nc.scalar.copy(img_t[:, 1:2], img_t[:, 3:4])
                    nc.scalar.copy(img_t[:, 0:1], img_t[:, 4:5])
                    nc.scalar.copy(img_t[:, W + 2:W + 3], img_t[:, W:W + 1])
                    nc.scalar.copy(img_t[:, W + 3:W + 4], img_t[:, W - 1:W])

                    # ---- quantize: xq = fp16(floor(255*img)) -------------------
                    t_int = mp.tile([128, W + 4], i32, tag="tint")
                    # floor(255*img) via rint(510*img - 0.5) >> 1 (tie-safe)
                    nc.vector.tensor_scalar(t_int[:, :], img_t[:, :], 510.0, -0.5,
                                            AL.mult, AL.add)
                    xq = mp.tile([128, W + 4], f16, tag="xq")  # col m = img col m-2
                    nc.vector.tensor_scalar(t_int[:, :], t_int[:, :], 1, None,
                                            AL.arith_shift_right)
                    nc.scalar.copy(xq[:, :], t_int[:, :])

                    # ---- blur: 5 dx x {fp16 hi + bf16 lo} band matmuls ---------
                    ps_blur = pp.tile([128, W], f32, tag="pblur")
                    for c0 in (0, 512):
                        for dxi, dx in enumerate((-2, -1, 0, 1, 2)):
                            rhs = xq[:, c0 + 2 + dx: c0 + 2 + dx + 512]
                            nc.tensor.matmul(ps_blur[:, c0:c0 + 512], m16(cls, dxi),
                                             rhs, start=(dxi == 0), stop=False)
                            nc.tensor.matmul(ps_blur[:, c0:c0 + 512], mlo(cls, dxi),
                                             rhs, start=False, stop=(dx == 2))

                    # ---- evacuate blur as exact fp16 hi/lo (lo via PE) ---------
                    bh = mp.tile([128, W + 2], f16, tag="bh")  # col m = blur col m-1
                    bl = mp.tile([128, W + 2], f16, tag="bl")
                    nc.scalar.copy(bh[:, 1:W + 1], ps_blur[:, :])
                    for c0 in (0, 512):  # psum -= I@bh  -> residual
                        nc.tensor.matmul(ps_blur[:, c0:c0 + 512], msh(2),
                                         bh[:, c0 + 1:c0 + 513], start=False,
                                         stop=True, skip_group_check=True)
                    nc.scalar.copy(bl[:, 1:W + 1], ps_blur[:, :])
                    for t in (bh, bl):  # blur col reflect: -1 = 1, 1024 = 1022
                        nc.scalar.copy(t[:, 0:1], t[:, 2:3])
                        nc.scalar.copy(t[:, W + 1:W + 2], t[:, W - 1:W])

                    # ---- sobel: gx (2 dx), gy (3 dx), each on {bh, bl} ---------
                    ps_gx = pp.tile([128, W], f32, tag="pgx")
                    ps_gy = pp.tile([128, W], f32, tag="pgy")
                    for c0 in (0, 512):
                        for ps, items in ((ps_gx, [(5, -1), (6, 1)]),
                                          (ps_gy, [(7, -1), (8, 0), (9, 1)])):
                            ops = []
                            for k, dx in items:
                                ops.append((k, bh, dx))
                                ops.append((k, bl, dx))
                            for n, (k, src, dx) in enumerate(ops):
                                nc.tensor.matmul(ps[:, c0:c0 + 512], m16(cls, k),
                                                 src[:, c0 + 1 + dx: c0 + 1 + dx + 512],
                                                 start=(n == 0), stop=(n == len(ops) - 1))

                    # ---- magnitude + bin code --------------------------------
                    gxs = mp.tile([128, W], f32, tag="gxs")
                    nc.scalar.copy(gxs[:, :], ps_gx[:, :])
                    mag = np_.tile([128, W + 2], f32, tag="mag")  # col m = img col m-1
                    nc.gpsimd.memset(mag[:, 0:1], 0.0)
                    nc.gpsimd.memset(mag[:, W + 1:W + 2], 0.0)
                    nc.vector._custom_dve(OP_MAG, out=mag[:, 1:W + 1], in0=gxs[:, :],
                                          in1=ps_gy[:, :])
                    posm = np_.tile([128, W], f32, tag="posm")
                    c0m = np_.tile([128, W], f32, tag="c0m")
                    c2m = np_.tile([128, W], f32, tag="c2m")
                    nc.vector._custom_dve(OP_POS, out=posm[:, :], in0=ps_gy[:, :],
                                          in1=gxs[:, :])
                    nc.vector._custom_dve(OP_C0, out=c0m[:, :], in0=ps_gy[:, :],
                                          in1=gxs[:, :], s0=T2)
                    nc.vector._custom_dve(OP_C2, out=c2m[:, :], in0=ps_gy[:, :],
                                          in1=gxs[:, :], s0=T2)

                    # ---- row shifts on PE (fp32 identity bands, exact) ---------
                    # dedicated 1-bank PSUM tiles (pu/pd) so the pgx/pgy slots
                    # free early and tile i+1's sobel overlaps tile i's tail
                    usb = np_.tile([128, W + 2], f32, tag="usb")
                    dsb = np_.tile([128, W + 2], f32, tag="dsb")
                    nc.gpsimd.memset(usb[:, 0:1], 0.0)
                    nc.gpsimd.memset(usb[:, W + 1:W + 2], 0.0)
                    nc.gpsimd.memset(dsb[:, 0:1], 0.0)
                    nc.gpsimd.memset(dsb[:, W + 1:W + 2], 0.0)
                    for c0 in (0, 512):
                        ps_u = pp.tile([128, 512], f32, tag="pu")
                        ps_d = pp.tile([128, 512], f32, tag="pd")
                        for (ps, k, dst) in ((ps_u, 0, usb), (ps_d, 1, dsb)):
                            nc.tensor.matmul(ps[:, :], msh32(k),
                                             mag[:, c0 + 1:c0 + 513],
                                             start=True, stop=True)
                            nc.scalar.copy(dst[:, c0 + 1:c0 + 513], ps[:, :])

                    # ---- NMS pair maxes + bin select + threshold ---------------
                    M = np_.tile([128, W], f32, tag="M")
                    mns = np_.tile([128, W], f32, tag="mns")
                    mnesw = np_.tile([128, W], f32, tag="mnesw")
                    mew = np_.tile([128, W], f32, tag="mew")
                    # default bin3 (NW/SE) in M; overwrite by pos/c2/c0 priority
                    nc.vector.tensor_tensor(M[:, :], usb[:, 0:W], dsb[:, 2:W + 2], AL.max)
                    nc.vector.tensor_tensor(mnesw[:, :], usb[:, 2:W + 2], dsb[:, 0:W], AL.max)
                    nc.vector.tensor_tensor(mns[:, :], usb[:, 1:W + 1], dsb[:, 1:W + 1], AL.max)
                    nc.vector.tensor_tensor(mew[:, :], mag[:, 0:W], mag[:, 2:W + 2], AL.max)
                    nc.vector.copy_predicated(M[:, :], posm.bitcast(i32)[:, :], mnesw[:, :])
                    nc.vector.copy_predicated(M[:, :], c2m.bitcast(i32)[:, :], mns[:, :])
                    nc.vector.copy_predicated(M[:, :], c0m.bitcast(i32)[:, :], mew[:, :])

                    keep = iop.tile([128, W], f32, tag="keep")
                    nc.vector._custom_dve(OP_KEEP, out=keep[:, :], in0=mag[:, 1:W + 1],
                                          in1=M[:, :], s0=THR)
                    nc.sync.dma_start(out=out_d[i, r0:r0 + NKEEP, :], in_=keep[0:NKEEP, :])

                    if dump and i == 0 and ti == tiles[0]:
                        for name, t in [("xq", xq), ("bh", bh), ("bl", bl),
                                        ("gxs", gxs), ("mag", mag), ("usb", usb),
                                        ("dsb", dsb), ("M", M), ("posm", posm)]:
                            fs = t.shape[1]
                            cvt = np_.tile([128, W + 4], f32, tag="cvt")
                            nc.vector.tensor_copy(cvt[:, 0:fs], t[:, :])
                            nc.sync.dma_start(out=dmp[name][:, 0:fs], in_=cvt[:, 0:fs])

    nc.compile()
    return nc


_CACHE = {}


def _get_kernel(n_img):
    key = n_img
    if key not in _CACHE:
        _CACHE[key] = (build_kernel(n_img), *_pack_weights())
    return _CACHE[key]


def kernel(image: np.ndarray) -> np.ndarray:
    image = np.ascontiguousarray(np.asarray(image, dtype=np.float32))
    b = image.shape[0]
    assert image.shape == (b, 1, H, W)
    n_cores = NCORES
    per = b // n_cores
    assert per * n_cores == b
    nc, wt16, wlo, wsh = _get_kernel(per)
    in_maps = []
    for c in range(n_cores):
        in_maps.append({
            "image": np.ascontiguousarray(image[c * per:(c + 1) * per, 0]),
            "wt16": wt16,
            "wlo": wlo,
            "wsh": wsh,
        })
    res = bass_utils.run_bass_kernel_spmd(nc, in_maps, core_ids=list(range(n_cores)))
    out = np.empty((b, 1, H, W), np.float32)
    for c in range(n_cores):
        out[c * per:(c + 1) * per, 0] = res.results[c]["out"]
    return out


# revision 16
# speedup vs baseline: 1.0808x; 1.0808x over previous
"""Canny edge detector (32,1,1024,1024) on 8 Trainium2 NeuronCores.

v2 strategy (per core: 4 images, data-parallel over batch):
  - Row-tiles of 128 partitions (120-row output tiles + 4-row halo wrap).
  - Blur: 5 dx-shifted band matmuls x {fp16 hi + bf16 lo} weight pair
    (bf16 lo needs no 2^12 rescale: bf16 exponent range covers ~1e-5
    residuals), rhs = fp16 xq for both. fp32 PSUM.
  - bh = fp16(blur) evac on ACT; bl computed on the PE (psum -= I@bh)
    and evac'd on ACT -> exact fp16 hi/lo pair without a DVE subtract.
  - Sobel: integer fp16 bands on {bh, bl}, PSUM fp32.
  - NMS: mag/direction-mask fused DVE customs reading gx(SBUF)/gy(PSUM);
    row-shifted U/D via fp32 identity-band matmuls into dedicated 1-bank
    PSUM tiles (pu/pd) so the blur/sobel PSUM slots free early and
    adjacent tiles overlap; pair maxes + 3 predicated overwrites +
    fused keep threshold on DVE.
  - Engine placement tuned to measured costs: ACT takes all evacuations
    and pad copies, DVE keeps only tensor-tensor/custom/predicated work.
"""

import os
import numpy as np

import concourse.bacc as bacc
import concourse.tile as tile
import concourse.mybir as mybir
from concourse import bass_utils
from concourse import dve_ops as _DO
from concourse.dve_spec import (
    Spec, Src0, Src1, C0, C1, C2, Zero, One, maxx, select, Bin, AluOp,
    lower as _dve_lower,
)
from concourse.dve_uop import DveOpSpec as _DveOpSpec


def _register_custom_op(name, body, ref):
    """Runtime-register a fused DVE op (sha self-computed, v3/TRN2)."""
    if name in _DO._SUB_OPCODE_FOR_NAME:
        return next(op for op in _DO.OPS if op.name == name)
    op = _DO.DveOp(name, Spec(body=body, reference=ref), subdim=False, uops_sha={})
    _DO.OPS.append(op)
    _DO.CUSTOM_DVE_SPECS[name] = op.spec
    _DO._SUB_OPCODE_FOR_NAME[name] = _DO._CUSTOM_DVE_ROW_BASE + len(_DO.OPS) - 1
    for ver in ("v3",):
        compiled = _DveOpSpec(
            name=name,
            opcode=_DO.get_dve_sub_opcode(name),
            uops=_dve_lower(op.spec, ver=ver),
            rd1_en=True,
        )
        op.uops_sha[ver] = compiled.sha(ver)
    return op


_ABS0 = maxx(Src0, Zero - Src0)
_ABS1 = maxx(Src1, Zero - Src1)
OP_MAG = _register_custom_op(
    "CANNY_MAG", _ABS0 + _ABS1,
    lambda in0, in1, s0, s1, imm2: np.abs(in0) + np.abs(in1))
# bin code: 0 = E/W (|gy|*T2 < |gx|), 2 = N/S (|gx|*T2 <= |gy|),
# 1 = NE/SW diag (gx*gy > 0), 3 = NW/SE diag. in0 = gx, in1 = gy.
OP_POS = _register_custom_op(
    "CANNY_POS", (Src0 * Src1) > Zero,
    lambda in0, in1, s0, s1, imm2: (in0 * in1 > 0).astype(np.float32))
OP_C0 = _register_custom_op(
    "CANNY_C0", (_ABS0 * C0) < _ABS1,
    lambda in0, in1, s0, s1, imm2: (np.abs(in0) * s0 < np.abs(in1)).astype(np.float32))
OP_C2 = _register_custom_op(
    "CANNY_C2", (_ABS1 * C0) <= _ABS0,
    lambda in0, in1, s0, s1, imm2: (np.abs(in1) * s0 <= np.abs(in0)).astype(np.float32))
OP_KEEP = _register_custom_op(
    "CANNY_KEEP", (Src0 >= Src1) & (Src0 > C0),
    lambda in0, in1, s0, s1, imm2: ((in0 >= in1) & (in0 > s0)).astype(np.float32))

H = W = 1024
NCORES = 8
IMGS_PER_CORE = 4
TILE_STARTS = [0, 120, 240, 360, 480, 600, 720, 840, 904]
NKEEP = 120
T2 = float(np.float32(1.0 + np.sqrt(2.0)))  # tan(67.5 deg)
THR = 20.0

# ----------------------------------------------------------------------------
# band construction (host, float64 -> float32 taps identical to the reference)
# ----------------------------------------------------------------------------

def _gauss5_f64():
    x = np.arange(5.0) - 2.0
    k = np.exp(-(x ** 2) / (2.0 * 9.0))
    return k / k.sum()

G64 = _gauss5_f64()
COL_SMOOTH = np.array([1.0, 2.0, 1.0])   # sobel smoothing (column filter of Sx)
COL_DIFF = np.array([-1.0, 0.0, 1.0])    # sobel derivative (column filter of Sy)


def _bf16(a):
    a32 = np.asarray(a, np.float32)
    u = a32.view(np.uint32)
    # round-to-nearest-even on the upper 16 bits
    u = (u + 0x7FFF + ((u >> 16) & 1)).astype(np.uint32) & np.uint32(0xFFFF0000)
    return u.view(np.float32)


def _row_of(p, r0):
    return r0 + (p if p < 124 else p - 128)


def _part_of(v, r0):
    d = v - r0
    assert -4 <= d < 124, (v, r0)
    return d if d >= 0 else d + 128


def _reflect(v):
    if v < 0:
        return -v
    if v > H - 1:
        return 2 * (H - 1) - v
    return v


def _blur_bands(r0):
    """(5 fp16 hi, 5 bf16-lo-as-f32) [128,128] matrices for dx=-2..2."""
    his, los = [], []
    for dx in range(-2, 3):
        B = np.zeros((128, 128), np.float64)
        for j in range(128):
            d = j if j < 124 else j - 128
            if not (-2 <= d <= 121):
                continue
            v = _row_of(j, r0)
            if not (0 <= v <= H - 1):
                continue
            for dz in range(-2, 3):
                u = v + dz  # virtual x row; tile holds r0-4..r0+123 virtually
                B[_part_of(u, r0), j] += G64[dz + 2] * G64[dx + 2]
        B32 = B.astype(np.float32)
        BH = B32.astype(np.float16)
        BL = _bf16(B32.astype(np.float64) - BH.astype(np.float64))
        his.append(BH)
        los.append(BL)
    return his, los


def _sobel_bands(r0):
    """5 fp16 [128,128] integer matrices: SGX(dx=-1), SGX(+1), SGY(-1), SGY(0), SGY(+1)."""
    mats = []
    for colfilt, rowtaps in ((COL_SMOOTH, [-1.0, 1.0]), (COL_DIFF, [1.0, 2.0, 1.0])):
        if colfilt is COL_SMOOTH:
            dxs = [-1, 1]
        else:
            dxs = [-1, 0, 1]
        for idx, dx in enumerate(dxs):
            rt = rowtaps[idx] if colfilt is COL_SMOOTH else rowtaps[dx + 1]
            B = np.zeros((128, 128), np.float64)
            for j in range(128):
                d = j if j < 124 else j - 128
                if not (-1 <= d <= 120):
                    continue
                v = _row_of(j, r0)
                if not (0 <= v <= H - 1):
                    continue
                for dz in (-1, 0, 1):
                    w = colfilt[dz + 1]
                    if w == 0.0:
                        continue
                    u = _reflect(v + dz)  # reflect-101 on BLUR rows
                    B[_part_of(u, r0), j] += rt * w
            mats.append(B.astype(np.float16))
    return mats


def _shift_mats():
    SUP = np.zeros((128, 128), np.float16)  # U[j] = mag[j-1 (mod 128)]
    SDN = np.zeros((128, 128), np.float16)  # D[j] = mag[j+1]
    NEGI = np.zeros((128, 128), np.float16)
    for j in range(128):
        SUP[(j - 1) % 128, j] = 1.0
        NEGI[j, j] = -1.0
    for j in range(127):
        SDN[j + 1, j] = 1.0
    return SUP, SDN, NEGI


def _pack_weights():
    """wt16 [128, 33*128] f16 (per class: 5 blur-hi + 5 sobel; then SUP, SDN, NEGI),
    wlo [128, 15*128] f32-encoded-bf16 column slices."""
    mats16 = []
    matslo = []
    for r0 in (TILE_STARTS[0], TILE_STARTS[1], TILE_STARTS[-1]):
        his, los = _blur_bands(r0)
        mats16.extend(his)
        mats16.extend(_sobel_bands(r0))
        matslo.extend(los)
    SUP, SDN, NEGI = _shift_mats()
    mats16.extend([SUP, SDN, NEGI])
    wt16 = np.stack([m.astype(np.float16) for m in mats16], 0)      # [33,128,128]
    wt16 = np.transpose(wt16, (1, 0, 2)).reshape(128, -1).copy()
    wlo32 = np.stack(matslo, 0)                                     # [15,128,128] f32
    wlo32 = np.transpose(wlo32, (1, 0, 2)).reshape(128, -1).copy()
    wsh = np.stack([SUP.astype(np.float32), SDN.astype(np.float32)], 0)
    wsh = np.transpose(wsh, (1, 0, 2)).reshape(128, -1).copy()
    import ml_dtypes
    return (wt16.astype(np.float16), wlo32.astype(ml_dtypes.bfloat16),
            wsh.astype(np.float32))


def _tile_class(ti):
    if ti == 0:
        return 0
    if ti == len(TILE_STARTS) - 1:
        return 2
    return 1


# ----------------------------------------------------------------------------
# kernel builder
# ----------------------------------------------------------------------------

def build_kernel(n_img=IMGS_PER_CORE, tiles=None, dump=False, repeat=1, ablate=()):
    if tiles is None:
        tiles = list(range(len(TILE_STARTS)))
    AL = mybir.AluOpType
    f32, f16, bf16, i32 = (mybir.dt.float32, mybir.dt.float16,
                           mybir.dt.bfloat16, mybir.dt.int32)

    nc = bacc.Bacc("TRN2", target_bir_lowering=False, debug=False)
    img_d = nc.dram_tensor("image", [n_img, H, W], f32, kind="ExternalInput").ap()
    wt16_d = nc.dram_tensor("wt16", [128, 33 * 128], f16, kind="ExternalInput").ap()
    wlo_d = nc.dram_tensor("wlo", [128, 15 * 128], bf16, kind="ExternalInput").ap()
    wsh_d = nc.dram_tensor("wsh", [128, 2 * 128], f32, kind="ExternalInput").ap()
    out_d = nc.dram_tensor("out", [n_img, H, W], f32, kind="ExternalOutput").ap()
    if dump:
        dmp = {k: nc.dram_tensor("dbg_" + k, [128, W + 4], f32, kind="ExternalOutput").ap()
               for k in ["xq", "bh", "bl", "gxs", "mag", "usb", "dsb", "M", "bin"]}

    with tile.TileContext(nc) as tc:
        with (
            tc.tile_pool(name="wts", bufs=1) as wp,
            tc.tile_pool(name="io", bufs=4) as iop,
            tc.tile_pool(name="mid", bufs=3) as mp,
            tc.tile_pool(name="nms", bufs=2) as np_,
            tc.tile_pool(name="ps", bufs=1, space="PSUM") as pp,
            tc.tile_pool(name="psA", bufs=2, space="PSUM") as ppA,
        ):
            wt16 = wp.tile([128, 33 * 128], f16)
            wlo = wp.tile([128, 15 * 128], bf16)
            wsh = wp.tile([128, 2 * 128], f32)
            nc.sync.dma_start(out=wt16[:, :], in_=wt16_d[:, :])
            nc.sync.dma_start(out=wlo[:, :], in_=wlo_d[:, :])
            nc.sync.dma_start(out=wsh[:, :], in_=wsh_d[:, :])

            def m16(c, k):  # fp16 matrix k (0..9) of tile-class c
                s = (c * 10 + k) * 128
                return wt16[:, s:s + 128]

            def msh(k):  # 0 = SUP, 1 = SDN, 2 = NEGI (fp16)
                s = (30 + k) * 128
                return wt16[:, s:s + 128]

            def msh32(k):  # 0 = SUP, 1 = SDN (fp32)
                return wsh[:, k * 128:(k + 1) * 128]

            def mlo(c, k):  # bf16 blur-lo matrix k (0..4) of tile-class c
                s = (c * 5 + k) * 128
                return wlo[:, s:s + 128]

            for _rep in range(repeat):
              for i in range(n_img):
                for ti in tiles:
                    r0 = TILE_STARTS[ti]
                    cls = _tile_class(ti)

                    # ---- load image tile (fp32), incl. reflected pad cols ------
                    img_t = iop.tile([128, W + 4], f32, tag="img")
                    if ti == len(TILE_STARTS) - 1:
                        nc.sync.dma_start(out=img_t[0:120, 2:W + 2],
                                          in_=img_d[i, r0:r0 + 120, :])
                        for k in range(4):  # virtual rows 1024..1027 = 1022..1019
                            nc.sync.dma_start(out=img_t[120 + k:121 + k, 2:W + 2],
                                              in_=img_d[i, 1022 - k:1023 - k, :])
                    else:
                        nc.sync.dma_start(out=img_t[0:124, 2:W + 2],
                                          in_=img_d[i, r0:r0 + 124, :])
                    if ti == 0:
                        for k in range(4):  # virtual rows -4..-1 = rows 4,3,2,1
                            nc.sync.dma_start(out=img_t[124 + k:125 + k, 2:W + 2],
                                              in_=img_d[i, 4 - k:5 - k, :])
                    else:
                        nc.sync.dma_start(out=img_t[124:128, 2:W + 2],
                                          in_=img_d[i, r0 - 4:r0, :])
                    # reflected pad columns (ACT: full 128-partition col copies)
                    nc.scalar.copy(img_t[:, 1:2], img_t[:, 3:4])
                    nc.scalar.copy(img_t[:, 0:1], img_t[:, 4:5])
                    nc.scalar.copy(img_t[:, W + 2:W + 3], img_t[:, W:W + 1])
                    nc.scalar.copy(img_t[:, W + 3:W + 4], img_t[:, W - 1:W])

                    # ---- quantize: xq = fp16(floor(255*img)) -------------------
                    t_int = mp.tile([128, W + 4], i32, tag="tint")
                    # floor(255*img) via rint(510*img - 0.5) >> 1 (tie-safe)
                    nc.vector.tensor_scalar(t_int[:, :], img_t[:, :], 510.0, -0.5,
                                            AL.mult, AL.add)
                    xq = mp.tile([128, W + 4], f16, tag="xq")  # col m = img col m-2
                    nc.vector.tensor_scalar(t_int[:, :], t_int[:, :], 1, None,
                                            AL.arith_shift_right)
                    nc.scalar.copy(xq[:, :], t_int[:, :])

                    # ---- blur + fp16 hi/lo evac, per 512-col half --------------
                    # 1-bank PSUM tiles from the double-buffered pool: half 1 of
                    # this tile and the blur of tile i+1 overlap this tile's tail
                    bh = mp.tile([128, W + 2], f16, tag="bh")  # col m = blur col m-1
                    bl = mp.tile([128, W + 2], f16, tag="bl")
                    for c0 in (0, 512):
                        ps_b = ppA.tile([128, 512], f32, tag="pblur")
                        for dxi, dx in enumerate((-2, -1, 0, 1, 2)):
                            rhs = xq[:, c0 + 2 + dx: c0 + 2 + dx + 512]
                            nc.tensor.matmul(ps_b[:, :], m16(cls, dxi),
                                             rhs, start=(dxi == 0), stop=False)
                            nc.tensor.matmul(ps_b[:, :], mlo(cls, dxi),
                                             rhs, start=False, stop=(dx == 2))
                        nc.scalar.copy(bh[:, c0 + 1:c0 + 513], ps_b[:, :])
                        nc.tensor.matmul(ps_b[:, :], msh(2), bh[:, c0 + 1:c0 + 513],
                                         start=False, stop=True, skip_group_check=True)
                        nc.scalar.copy(bl[:, c0 + 1:c0 + 513], ps_b[:, :])
                    for t in (bh, bl):  # blur col reflect: -1 = 1, 1024 = 1022
                        nc.scalar.copy(t[:, 0:1], t[:, 2:3])
                        nc.scalar.copy(t[:, W + 1:W + 2], t[:, W - 1:W])

                    # ---- sobel + mag/masks, per 512-col half -------------------
                    gxs = mp.tile([128, W], f32, tag="gxs")
                    mag = np_.tile([128, W + 2], f32, tag="mag")  # col m = img col m-1
                    nc.gpsimd.memset(mag[:, 0:1], 0.0)
                    nc.gpsimd.memset(mag[:, W + 1:W + 2], 0.0)
                    posm = np_.tile([128, W], f32, tag="posm")
                    c0m = np_.tile([128, W], f32, tag="c0m")
                    c2m = np_.tile([128, W], f32, tag="c2m")
                    for c0 in (0, 512):
                        ps_gx = ppA.tile([128, 512], f32, tag="pgx")
                        ps_gy = ppA.tile([128, 512], f32, tag="pgy")
                        for ps, items in ((ps_gx, [(5, -1), (6, 1)]),
                                          (ps_gy, [(7, -1), (8, 0), (9, 1)])):
                            ops = []
                            for k, dx in items:
                                ops.append((k, bh, dx))
                                ops.append((k, bl, dx))
                            for n, (k, src, dx) in enumerate(ops):
                                nc.tensor.matmul(ps[:, :], m16(cls, k),
                                                 src[:, c0 + 1 + dx: c0 + 1 + dx + 512],
                                                 start=(n == 0), stop=(n == len(ops) - 1))
                        nc.scalar.copy(gxs[:, c0:c0 + 512], ps_gx[:, :])
                        nc.vector._custom_dve(OP_MAG, out=mag[:, c0 + 1:c0 + 513],
                                              in0=gxs[:, c0:c0 + 512], in1=ps_gy[:, :])
                        nc.vector._custom_dve(OP_POS, out=posm[:, c0:c0 + 512],
                                              in0=ps_gy[:, :], in1=gxs[:, c0:c0 + 512])
                        nc.vector._custom_dve(OP_C0, out=c0m[:, c0:c0 + 512],
                                              in0=ps_gy[:, :], in1=gxs[:, c0:c0 + 512],
                                              s0=T2)
                        nc.vector._custom_dve(OP_C2, out=c2m[:, c0:c0 + 512],
                                              in0=ps_gy[:, :], in1=gxs[:, c0:c0 + 512],
                                              s0=T2)

                    # ---- row shifts on PE (fp32 identity bands, exact) ---------
                    # dedicated 1-bank PSUM tiles (pu/pd) so the pgx/pgy slots
                    # free early and tile i+1's sobel overlaps tile i's tail
                    usb = np_.tile([128, W + 2], f32, tag="usb")
                    dsb = np_.tile([128, W + 2], f32, tag="dsb")
                    nc.gpsimd.memset(usb[:, 0:1], 0.0)
                    nc.gpsimd.memset(usb[:, W + 1:W + 2], 0.0)
                    nc.gpsimd.memset(dsb[:, 0:1], 0.0)
                    nc.gpsimd.memset(dsb[:, W + 1:W + 2], 0.0)
                    for c0 in (0, 512):
                        ps_u = pp.tile([128, 512], f32, tag="pu")
                        ps_d = pp.tile([128, 512], f32, tag="pd")
                        for (ps, k, dst) in ((ps_u, 0, usb), (ps_d, 1, dsb)):
                            nc.tensor.matmul(ps[:, :], msh32(k),
                                             mag[:, c0 + 1:c0 + 513],
                                             start=True, stop=True)
                            nc.scalar.copy(dst[:, c0 + 1:c0 + 513], ps[:, :])

                    # ---- NMS pair maxes + bin select + threshold ---------------
                    M = np_.tile([128, W], f32, tag="M")
                    mns = np_.tile([128, W], f32, tag="mns")
                    mnesw = np_.tile([128, W], f32, tag="mnesw")
                    mew = np_.tile([128, W], f32, tag="mew")
                    # default bin3 (NW/SE) in M; overwrite by pos/c2/c0 priority
                    nc.vector.tensor_tensor(M[:, :], usb[:, 0:W], dsb[:, 2:W + 2], AL.max)
                    nc.vector.tensor_tensor(mnesw[:, :], usb[:, 2:W + 2], dsb[:, 0:W], AL.max)
                    nc.vector.tensor_tensor(mns[:, :], usb[:, 1:W + 1], dsb[:, 1:W + 1], AL.max)
                    nc.vector.tensor_tensor(mew[:, :], mag[:, 0:W], mag[:, 2:W + 2], AL.max)
                    nc.vector.copy_predicated(M[:, :], posm.bitcast(i32)[:, :], mnesw[:, :])
                    nc.vector.copy_predicated(M[:, :], c2m.bitcast(i32)[:, :], mns[:, :])
                    nc.vector.copy_predicated(M[:, :], c0m.bitcast(i32)[:, :], mew[:, :])

                    keep = iop.tile([128, W], f32, tag="keep")
                    nc.vector._custom_dve(OP_KEEP, out=keep[:, :], in0=mag[:, 1:W + 1],
                                          in1=M[:, :], s0=THR)
                    nc.sync.dma_start(out=out_d[i, r0:r0 + NKEEP, :], in_=keep[0:NKEEP, :])

                    if dump and i == 0 and ti == tiles[0]:
                        for name, t in [("xq", xq), ("bh", bh), ("bl", bl),
                                        ("gxs", gxs), ("mag", mag), ("usb", usb),
                                        ("dsb", dsb), ("M", M), ("posm", posm)]:
                            fs = t.shape[1]
                            cvt = np_.tile([128, W + 4], f32, tag="cvt")
                            nc.vector.tensor_copy(cvt[:, 0:fs], t[:, :])
                            nc.sync.dma_start(out=dmp[name][:, 0:fs], in_=cvt[:, 0:fs])

    nc.compile()
    return nc


_CACHE = {}


def _get_kernel(n_img):
    key = n_img
    if key not in _CACHE:
        _CACHE[key] = (build_kernel(n_img), *_pack_weights())
    return _CACHE[key]


def kernel(image: np.ndarray) -> np.ndarray:
    image = np.ascontiguousarray(np.asarray(image, dtype=np.float32))
    b = image.shape[0]
    assert image.shape == (b, 1, H, W)
    n_cores = NCORES
    per = b // n_cores
    assert per * n_cores == b
    nc, wt16, wlo, wsh = _get_kernel(per)
    in_maps = []
    for c in range(n_cores):
        in_maps.append({
            "image": np.ascontiguousarray(image[c * per:(c + 1) * per, 0]),
            "wt16": wt16,
            "wlo": wlo,
            "wsh": wsh,
        })
    res = bass_utils.run_bass_kernel_spmd(nc, in_maps, core_ids=list(range(n_cores)))
    out = np.empty((b, 1, H, W), np.float32)
    for c in range(n_cores):
        out[c * per:(c + 1) * per, 0] = res.results[c]["out"]
    return out
